# revision 19
# baseline (speedup 1.0000x reference)
"""Trainium2 Bass kernel for nn_Alignment.

Per batch b (32 independent blocks):
    a_out = relu(a_in @ W1 + b1)          [512, 768]
    b_out = relu(b_in @ W2 + b2)          [512, 768]
    S     = (a_out @ b_out.T) * temp      [512(s), 512(t)]
    a_att = softmax(S, axis=s);  b_att = softmax(S, axis=t)
    a_feature = a_att.T @ a_in            [512(t), 1536]
    b_feature = b_att @ b_in              [512(s), 1536]

Key structure: both softmaxes share one exp(temp*S); only the
normalizers differ (col-sums for a_att, row-sums for b_att).  The sums
come free via the ScalarE activation accum_out, and each normalizer is
a per-partition scalar folded into the PSUM->SBUF epilogue of the
corresponding feature matmul.

Precision plan: the projection and scores matmuls run in fp8e4m3 with
perf_mode=DoubleRow (2 contraction elements per PE cell per cycle,
~1.4x bf16 rate at FD=512); weights are pre-scaled by 2^13 host-side so
their [-0.026, 0.026] range uses e4m3 normals, and the projection
epilogue folds the 2^-13 dequant (and a 32x requant for the fp8 score
operands) into the ScalarE Relu.  The feature matmuls -- whose operands
directly weight the output -- stay bf16 (fp32 accumulation in PSUM).
The E-transpose needed by b_feature runs on the PE against a bf16
identity (full-rate, exact).

Sharding: data-parallel over batch -- 4 batches per core on 8 cores,
weights replicated.  Masks are all-ones per the problem spec (mask==1
makes the reference exactly maskless), so they do not enter the device
program.
"""

import functools
from contextlib import ExitStack

import ml_dtypes
import numpy as np

import concourse.tile as tile
from concourse import bacc
from concourse import mybir
from concourse.bass_utils import run_bass_kernel_spmd
from concourse.masks import make_identity

FP32 = mybir.dt.float32
FP16 = mybir.dt.float16
BF16 = mybir.dt.bfloat16
FP8 = mybir.dt.float8e4
AFT = mybir.ActivationFunctionType
DR = mybir.MatmulPerfMode.DoubleRow

B, L, EH, H = 32, 512, 1536, 768
N_CORES = 8
BPC = B // N_CORES  # batches per core
P = 128
SI = L // P    # 4  seq partition tiles
EJ = EH // P   # 12 embedding partition tiles
HM = H // P    # 6  hidden partition tiles
NF = 512       # matmul free-dim chunk (one PSUM bank of fp32)
NJ = EH // NF  # 3  feature free chunks

WSCALE = 2.0 ** 13   # host-side weight pre-scale into e4m3 range
OSCALE = 32.0        # fp8 projection-output requant scale


def _maybe_loop(tc, repeat: int):
    import contextlib
    if repeat <= 1:
        return contextlib.nullcontext()
    return tc.For_i(0, repeat, 1,
                    hint_engines=(mybir.EngineType.PE, mybir.EngineType.DVE,
                                  mybir.EngineType.Activation, mybir.EngineType.SP))


def _build(temp: float, repeat: int = 1, body: int = 1) -> bacc.Bacc:
    nc = bacc.Bacc("TRN2", target_bir_lowering=False)
    a_in = nc.dram_tensor("a_bf", [BPC, L, EH], BF16, kind="ExternalInput").ap()
    b_in = nc.dram_tensor("b_bf", [BPC, L, EH], BF16, kind="ExternalInput").ap()
    # host-pretransposed fp8 projection operands: [b, ej, p, s]
    a_t8 = nc.dram_tensor("a8t", [BPC, EJ, P, L], FP8, kind="ExternalInput").ap()
    b_t8 = nc.dram_tensor("b8t", [BPC, EJ, P, L], FP8, kind="ExternalInput").ap()
    W1 = nc.dram_tensor("W1q", [EH, H], FP8, kind="ExternalInput").ap()
    b1 = nc.dram_tensor("b1s", [H], FP32, kind="ExternalInput").ap()
    W2 = nc.dram_tensor("W2q", [EH, H], FP8, kind="ExternalInput").ap()
    b2 = nc.dram_tensor("b2s", [H], FP32, kind="ExternalInput").ap()
    # f16 outputs (host upcasts): halves store DMA vs f32; f16's 10 mantissa
    # bits add only ~0.05% element error on top of the fp8 budget.
    a_ft = nc.dram_tensor("a_feature", [BPC, L, EH], FP16, kind="ExternalOutput").ap()
    b_ft = nc.dram_tensor("b_feature", [BPC, L, EH], FP16, kind="ExternalOutput").ap()

    # exp(temp*S) computed from the 32x-scaled fp8 score operands
    exp_scale = temp / (OSCALE * OSCALE)
    # projection epilogue: a8 = OSCALE*relu(psum/WSCALE + b) = relu(psum*OSCALE/WSCALE + OSCALE*b)
    proj_scale = OSCALE / WSCALE

    with tile.TileContext(nc) as tc, ExitStack() as ctx:
        consts = ctx.enter_context(tc.tile_pool(name="consts", bufs=1))
        big = ctx.enter_context(tc.tile_pool(name="big", bufs=4))
        t8buf = ctx.enter_context(tc.tile_pool(name="t8buf", bufs=2))
        proj = ctx.enter_context(tc.tile_pool(name="proj", bufs=1))
        epool = ctx.enter_context(tc.tile_pool(name="epool", bufs=1))
        sums = ctx.enter_context(tc.tile_pool(name="sums", bufs=2))
        outp = ctx.enter_context(tc.tile_pool(name="outp", bufs=4))
        ps1 = ctx.enter_context(tc.tile_pool(name="ps1", bufs=3, space="PSUM"))
        ps3 = ctx.enter_context(tc.tile_pool(name="ps3", bufs=5, space="PSUM"))

        ident = consts.tile([P, P], BF16)
        make_identity(nc, ident)

        def load_side(ib, side, x_dram):
            """DMA one batch side bf16 -> SBUF natural layout [P, SI, EH]."""
            xb = big.tile([P, SI, EH], BF16, tag=f"{side}_bf")
            for si in range(SI):
                nc.sync.dma_start(
                    out=xb[:, si, :],
                    in_=x_dram[ib, si * P:(si + 1) * P, :])
            return xb

        def load_t8(ib, side, t8_dram):
            """Host-pretransposed fp8 [EJ, P, L] -> SBUF [P, EJ, L]."""
            x8 = t8buf.tile([P, EJ, L], FP8, tag=f"{side}t_8")
            nc.sync.dma_start(
                out=x8, in_=t8_dram[ib].rearrange("ej p s -> p ej s"))
            return x8

        def emit_weight(name, w):
            # fp8 weights, partition-tiled over EH: [P, EJ, H]; natural
            # layout is already the projection lhsT.
            wt = consts.tile([P, EJ, H], FP8, name=name, tag=name)
            nc.sync.dma_start(out=wt, in_=w.rearrange("(ko p) h -> p ko h", p=P))
            return wt

        def emit_bias(name, bvec):
            # [H] -> [P, HM] with bt[p, j] = b[j*P + p].
            bt = consts.tile([P, HM], FP32, name=name, tag=name)
            nc.sync.dma_start(out=bt, in_=bvec.rearrange("(j p) -> p j", p=P))
            return bt

        # Startup order: a-side data first (feeds the PE batch-0
        # transposes), then W1 (feeds proj-a right when the transposes
        # finish), then b-side, then W2.
        preloaded = {}
        if repeat == 1:
            preloaded[(0, "a")] = load_side(0, "a", a_in)
            w1_q = emit_weight("w1", W1)
            preloaded[(0, "b")] = load_side(0, "b", b_in)
            w2_q = emit_weight("w2", W2)
        else:
            w1_q = emit_weight("w1", W1)
            w2_q = emit_weight("w2", W2)
        b1_t = emit_bias("b1t", b1)
        b2_t = emit_bias("b2t", b2)

        # repeat>1 wraps the whole per-core compute in a hardware
        # loop (timing harness; identical work each iteration).
        with _maybe_loop(tc, repeat):
          for _body in range(body):
            for ib in range(BPC):
                # ---- load natural bf16 + host-pretransposed fp8 ---------
                x_bf = {}   # side -> natural [P, SI, EH] bf16
                xt_8 = {}   # side -> transposed [P, EJ, L] fp8
                for side, x_dram, t8_dram in (("a", a_in, a_t8),
                                              ("b", b_in, b_t8)):
                    xb = preloaded.pop((ib, side), None)
                    if xb is None:
                        xb = load_side(ib, side, x_dram)
                    x_bf[side] = xb
                    xt_8[side] = load_t8(ib, side, t8_dram)

                # ---- projections (fp8 DoubleRow):
                #      out8[h, s] = OSCALE*relu(W.T @ x.T / WSCALE + b) ----
                outT = {}
                for side, wt, bt in (("a", w1_q, b1_t), ("b", w2_q, b2_t)):
                    ot = proj.tile([P, HM, L], FP8, tag=f"{side}_outT")
                    xt = xt_8[side]
                    for hm in range(HM):
                        pt = ps1.tile([P, L], FP32, tag="ps1")
                        for ek in range(0, EJ, 2):
                            nc.tensor.matmul(
                                pt,
                                lhsT=wt[:, ek:ek + 2, hm * P:(hm + 1) * P],
                                rhs=xt[:, ek:ek + 2, :],
                                start=(ek == 0), stop=(ek == EJ - 2),
                                perf_mode=DR,
                            )
                        nc.scalar.activation(
                            out=ot[:, hm, :], in_=pt,
                            func=AFT.Relu, bias=bt[:, hm:hm + 1],
                            scale=proj_scale,
                        )
                    outT[side] = ot

                # ---- scores (fp8 DoubleRow) + shared exp;
                #      row-sums via accum_out ----------------------------
                ea = epool.tile([P, SI, L], BF16, tag="ea")        # E[s, t]
                rowsum = sums.tile([P, SI], FP32, tag="rowsum")
                for sm in range(SI):
                    pt = ps1.tile([P, L], FP32, tag="ps1")
                    for hk in range(0, HM, 2):
                        nc.tensor.matmul(
                            pt,
                            lhsT=outT["a"][:, hk:hk + 2, sm * P:(sm + 1) * P],
                            rhs=outT["b"][:, hk:hk + 2, :],
                            start=(hk == 0), stop=(hk == HM - 2),
                            perf_mode=DR,
                        )
                    nc.scalar.activation(out=ea[:, sm, :], in_=pt,
                                         func=AFT.Exp, scale=exp_scale,
                                         accum_out=rowsum[:, sm:sm + 1])
                rrow = sums.tile([P, SI], FP32, tag="rrow")
                nc.vector.reciprocal(out=rrow, in_=rowsum)

                # ---- transpose E (PE, bf16); col-sums via accum_out -----
                eat = epool.tile([P, SI, L], BF16, tag="eat")      # E[t, s]
                colsum = sums.tile([P, SI], FP32, tag="colsum")
                for tm in range(SI):
                    pt = ps1.tile([P, L], FP32, tag="ps1")
                    for sk in range(SI):
                        nc.tensor.matmul(
                            pt[:, sk * P:(sk + 1) * P],
                            lhsT=ea[:, sk, tm * P:(tm + 1) * P],
                            rhs=ident, start=True, stop=True,
                        )
                    nc.scalar.activation(out=eat[:, tm, :], in_=pt,
                                         func=AFT.Copy,
                                         accum_out=colsum[:, tm:tm + 1])
                rcol = sums.tile([P, SI], FP32, tag="rcol")
                nc.vector.reciprocal(out=rcol, in_=colsum)

                # ---- features, a/b groups interleaved so the DVE
                #      epilogues stream while the PE fills the next bank --
                #      a_feature[t, e] = (E.T @ a_nat)[t, e] / colsum[t]
                #      b_feature[s, e] = (E @ b_nat)[s, e] / rowsum[s]
                for fm in range(SI):
                    for lhs, rvec, x_nat, ft in (
                        (ea, rcol, x_bf["a"], a_ft),
                        (eat, rrow, x_bf["b"], b_ft),
                    ):
                        pts = [ps3.tile([P, NF], FP32, tag="ps3", name=f"psf{nj}") for nj in range(NJ)]
                        for sk in range(SI):
                            for nj in range(NJ):
                                nc.tensor.matmul(
                                    pts[nj],
                                    lhsT=lhs[:, sk, fm * P:(fm + 1) * P],
                                    rhs=x_nat[:, sk, nj * NF:(nj + 1) * NF],
                                    start=(sk == 0), stop=(sk == SI - 1),
                                )
                        ot = outp.tile([P, EH], FP16, tag="out")
                        for nj in range(NJ):
                            nc.vector.tensor_scalar_mul(
                                out=ot[:, nj * NF:(nj + 1) * NF],
                                in0=pts[nj], scalar1=rvec[:, fm:fm + 1])
                        nc.sync.dma_start(
                            out=ft[ib, fm * P:(fm + 1) * P, :], in_=ot)

    nc.compile()
    return nc


@functools.lru_cache(maxsize=4)
def _build_cached(temp: float, repeat: int = 1, body: int = 1) -> bacc.Bacc:
    return _build(temp, repeat, body)


def _prep_inputs(inputs: dict) -> list[dict]:
    a_f32 = np.asarray(inputs["a_inputs"], dtype=np.float32)
    b_f32 = np.asarray(inputs["b_inputs"], dtype=np.float32)
    a_bf = np.ascontiguousarray(a_f32.astype(ml_dtypes.bfloat16))
    b_bf = np.ascontiguousarray(b_f32.astype(ml_dtypes.bfloat16))

    def t8(x):
        # [B, L, EH] -> fp8 [B, EJ, P, L] with [b, ej, p, s] = x[b, s, ej*P+p]
        q = x.astype(ml_dtypes.float8_e4m3)
        return np.ascontiguousarray(
            q.reshape(B, L, EJ, P).transpose(0, 2, 3, 1))

    a8t = t8(a_f32)
    b8t = t8(b_f32)
    W1q = np.ascontiguousarray(np.clip(
        np.asarray(inputs["W1"], dtype=np.float32) * WSCALE, -240, 240
    ).astype(ml_dtypes.float8_e4m3))
    W2q = np.ascontiguousarray(np.clip(
        np.asarray(inputs["W2"], dtype=np.float32) * WSCALE, -240, 240
    ).astype(ml_dtypes.float8_e4m3))
    b1s = np.ascontiguousarray(np.asarray(inputs["b1"], np.float32) * OSCALE)
    b2s = np.ascontiguousarray(np.asarray(inputs["b2"], np.float32) * OSCALE)

    in_maps = []
    for c in range(N_CORES):
        sl = slice(c * BPC, (c + 1) * BPC)
        in_maps.append({
            "a_bf": a_bf[sl],
            "b_bf": b_bf[sl],
            "a8t": a8t[sl],
            "b8t": b8t[sl],
            "W1q": W1q, "b1s": b1s, "W2q": W2q, "b2s": b2s,
        })
    return in_maps


def _run(inputs: dict, trace: bool = False):
    temp = float(np.asarray(inputs["temperature"]))
    nc = _build_cached(temp)
    in_maps = _prep_inputs(inputs)
    res = run_bass_kernel_spmd(nc, in_maps, list(range(N_CORES)), trace=trace)
    a_feat = np.concatenate(
        [res.results[c]["a_feature"].astype(np.float32) for c in range(N_CORES)], axis=0)
    b_feat = np.concatenate(
        [res.results[c]["b_feature"].astype(np.float32) for c in range(N_CORES)], axis=0)
    return (a_feat, b_feat), res


def kernel(a_inputs, a_mask, b_inputs, b_mask, W1, b1, W2, b2, temperature):
    (a_feat, b_feat), _ = _run({
        "a_inputs": a_inputs, "b_inputs": b_inputs,
        "W1": W1, "b1": b1, "W2": W2, "b2": b2,
        "temperature": temperature,
    })
    return (a_feat, b_feat)


# revision 20
# speedup vs baseline: 1.0576x; 1.0576x over previous
"""Trainium2 Bass kernel for nn_Alignment.

Per batch b (32 independent blocks):
    a_out = relu(a_in @ W1 + b1)          [512, 768]
    b_out = relu(b_in @ W2 + b2)          [512, 768]
    S     = (a_out @ b_out.T) * temp      [512(s), 512(t)]
    a_att = softmax(S, axis=s);  b_att = softmax(S, axis=t)
    a_feature = a_att.T @ a_in            [512(t), 1536]
    b_feature = b_att @ b_in              [512(s), 1536]

Key structure: both softmaxes share one exp(temp*S); only the
normalizers differ (col-sums for a_att, row-sums for b_att).  The sums
come free via the ScalarE activation accum_out, and each normalizer is
a per-partition scalar folded into the PSUM->SBUF epilogue of the
corresponding feature matmul.

Precision plan: the projection and scores matmuls run in fp8e4m3 with
perf_mode=DoubleRow (2 contraction elements per PE cell per cycle,
~1.4x bf16 rate at FD=512); weights are pre-scaled by 2^13 host-side so
their [-0.026, 0.026] range uses e4m3 normals, and the projection
epilogue folds the 2^-13 dequant (and a 32x requant for the fp8 score
operands) into the ScalarE Relu.  The feature matmuls -- whose operands
directly weight the output -- stay bf16 (fp32 accumulation in PSUM).
The E-transpose needed by b_feature runs on the PE against a bf16
identity (full-rate, exact).

Sharding: data-parallel over batch -- 4 batches per core on 8 cores,
weights replicated.  Masks are all-ones per the problem spec (mask==1
makes the reference exactly maskless), so they do not enter the device
program.
"""

import functools
from contextlib import ExitStack

import ml_dtypes
import numpy as np

import concourse.tile as tile
from concourse import bacc
from concourse import mybir
from concourse.bass_utils import run_bass_kernel_spmd
from concourse.masks import make_identity

FP32 = mybir.dt.float32
FP16 = mybir.dt.float16
BF16 = mybir.dt.bfloat16
FP8 = mybir.dt.float8e4
AFT = mybir.ActivationFunctionType
DR = mybir.MatmulPerfMode.DoubleRow

B, L, EH, H = 32, 512, 1536, 768
N_CORES = 8
BPC = B // N_CORES  # batches per core
P = 128
SI = L // P    # 4  seq partition tiles
EJ = EH // P   # 12 embedding partition tiles
HM = H // P    # 6  hidden partition tiles
NF = 512       # matmul free-dim chunk (one PSUM bank of fp32)
NJ = EH // NF  # 3  feature free chunks

WSCALE = 2.0 ** 13   # host-side weight pre-scale into e4m3 range
OSCALE = 32.0        # fp8 projection-output requant scale


def _maybe_loop(tc, repeat: int):
    import contextlib
    if repeat <= 1:
        return contextlib.nullcontext()
    return tc.For_i(0, repeat, 1,
                    hint_engines=(mybir.EngineType.PE, mybir.EngineType.DVE,
                                  mybir.EngineType.Activation, mybir.EngineType.SP))


def _build(temp: float, repeat: int = 1, body: int = 1) -> bacc.Bacc:
    nc = bacc.Bacc("TRN2", target_bir_lowering=False)
    a_in = nc.dram_tensor("a_bf", [BPC, L, EH], BF16, kind="ExternalInput").ap()
    b_in = nc.dram_tensor("b_bf", [BPC, L, EH], BF16, kind="ExternalInput").ap()
    # host-pretransposed fp8 projection operands: [b, ej, p, s]
    a_t8 = nc.dram_tensor("a8t", [BPC, EJ, P, L], FP8, kind="ExternalInput").ap()
    b_t8 = nc.dram_tensor("b8t", [BPC, EJ, P, L], FP8, kind="ExternalInput").ap()
    W1 = nc.dram_tensor("W1q", [EH, H], FP8, kind="ExternalInput").ap()
    b1 = nc.dram_tensor("b1s", [H], FP32, kind="ExternalInput").ap()
    W2 = nc.dram_tensor("W2q", [EH, H], FP8, kind="ExternalInput").ap()
    b2 = nc.dram_tensor("b2s", [H], FP32, kind="ExternalInput").ap()
    # f16 outputs (host upcasts): halves store DMA vs f32; f16's 10 mantissa
    # bits add only ~0.05% element error on top of the fp8 budget.
    a_ft = nc.dram_tensor("a_feature", [BPC, L, EH], FP16, kind="ExternalOutput").ap()
    b_ft = nc.dram_tensor("b_feature", [BPC, L, EH], FP16, kind="ExternalOutput").ap()

    # exp(temp*S) computed from the 32x-scaled fp8 score operands
    exp_scale = temp / (OSCALE * OSCALE)
    # projection epilogue: a8 = OSCALE*relu(psum/WSCALE + b) = relu(psum*OSCALE/WSCALE + OSCALE*b)
    proj_scale = OSCALE / WSCALE

    with tile.TileContext(nc) as tc, ExitStack() as ctx:
        consts = ctx.enter_context(tc.tile_pool(name="consts", bufs=1))
        big = ctx.enter_context(tc.tile_pool(name="big", bufs=4))
        t8buf = ctx.enter_context(tc.tile_pool(name="t8buf", bufs=2))
        proj = ctx.enter_context(tc.tile_pool(name="proj", bufs=1))
        epool = ctx.enter_context(tc.tile_pool(name="epool", bufs=1))
        sums = ctx.enter_context(tc.tile_pool(name="sums", bufs=2))
        outp = ctx.enter_context(tc.tile_pool(name="outp", bufs=4))
        ps1 = ctx.enter_context(tc.tile_pool(name="ps1", bufs=3, space="PSUM"))
        ps3 = ctx.enter_context(tc.tile_pool(name="ps3", bufs=5, space="PSUM"))

        ident = consts.tile([P, P], BF16)
        make_identity(nc, ident)

        def load_side(ib, side, x_dram):
            """DMA one batch side bf16 -> SBUF natural layout [P, SI, EH]."""
            xb = big.tile([P, SI, EH], BF16, tag=f"{side}_bf")
            for si in range(SI):
                nc.sync.dma_start(
                    out=xb[:, si, :],
                    in_=x_dram[ib, si * P:(si + 1) * P, :])
            return xb

        def load_t8(ib, side, t8_dram):
            """Host-pretransposed fp8 [EJ, P, L] -> SBUF [P, EJ, L]."""
            x8 = t8buf.tile([P, EJ, L], FP8, tag=f"{side}t_8")
            nc.sync.dma_start(
                out=x8, in_=t8_dram[ib].rearrange("ej p s -> p ej s"))
            return x8

        def emit_weight(name, w):
            # fp8 weights, partition-tiled over EH: [P, EJ, H]; natural
            # layout is already the projection lhsT.
            wt = consts.tile([P, EJ, H], FP8, name=name, tag=name)
            nc.sync.dma_start(out=wt, in_=w.rearrange("(ko p) h -> p ko h", p=P))
            return wt

        def emit_bias(name, bvec):
            # [H] -> [P, HM] with bt[p, j] = b[j*P + p].
            bt = consts.tile([P, HM], FP32, name=name, tag=name)
            nc.sync.dma_start(out=bt, in_=bvec.rearrange("(j p) -> p j", p=P))
            return bt

        # Startup order: a-side data first (feeds the PE batch-0
        # transposes), then W1 (feeds proj-a right when the transposes
        # finish), then b-side, then W2.
        preloaded = {}
        if repeat == 1:
            preloaded[(0, "a")] = load_side(0, "a", a_in)
            w1_q = emit_weight("w1", W1)
            preloaded[(0, "b")] = load_side(0, "b", b_in)
            w2_q = emit_weight("w2", W2)
        else:
            w1_q = emit_weight("w1", W1)
            w2_q = emit_weight("w2", W2)
        b1_t = emit_bias("b1t", b1)
        b2_t = emit_bias("b2t", b2)

        # repeat>1 wraps the whole per-core compute in a hardware
        # loop (timing harness; identical work each iteration).
        with _maybe_loop(tc, repeat):
          for _body in range(body):
            for ib in range(BPC):
                # ---- load natural bf16 + host-pretransposed fp8 ---------
                x_bf = {}   # side -> natural [P, SI, EH] bf16
                xt_8 = {}   # side -> transposed [P, EJ, L] fp8
                for side, x_dram, t8_dram in (("a", a_in, a_t8),
                                              ("b", b_in, b_t8)):
                    xb = preloaded.pop((ib, side), None)
                    if xb is None:
                        xb = load_side(ib, side, x_dram)
                    x_bf[side] = xb
                    xt_8[side] = load_t8(ib, side, t8_dram)

                # ---- projections (fp8 DoubleRow):
                #      out8[h, s] = OSCALE*relu(W.T @ x.T / WSCALE + b) ----
                outT = {}
                for side, wt, bt in (("a", w1_q, b1_t), ("b", w2_q, b2_t)):
                    ot = proj.tile([P, HM, L], FP8, tag=f"{side}_outT")
                    xt = xt_8[side]
                    for hm in range(HM):
                        pt = ps1.tile([P, L], FP32, tag="ps1")
                        for ek in range(0, EJ, 2):
                            nc.tensor.matmul(
                                pt,
                                lhsT=wt[:, ek:ek + 2, hm * P:(hm + 1) * P],
                                rhs=xt[:, ek:ek + 2, :],
                                start=(ek == 0), stop=(ek == EJ - 2),
                                perf_mode=DR,
                            )
                        nc.scalar.activation(
                            out=ot[:, hm, :], in_=pt,
                            func=AFT.Relu, bias=bt[:, hm:hm + 1],
                            scale=proj_scale,
                        )
                    outT[side] = ot

                # ---- scores (fp8 DoubleRow) + shared exp;
                #      row-sums via accum_out ----------------------------
                ea = epool.tile([P, SI, L], BF16, tag="ea")        # E[s, t]
                rowsum = sums.tile([P, SI], FP32, tag="rowsum")
                for sm in range(SI):
                    pt = ps1.tile([P, L], FP32, tag="ps1")
                    for hk in range(0, HM, 2):
                        nc.tensor.matmul(
                            pt,
                            lhsT=outT["a"][:, hk:hk + 2, sm * P:(sm + 1) * P],
                            rhs=outT["b"][:, hk:hk + 2, :],
                            start=(hk == 0), stop=(hk == HM - 2),
                            perf_mode=DR,
                        )
                    nc.scalar.activation(out=ea[:, sm, :], in_=pt,
                                         func=AFT.Exp, scale=exp_scale,
                                         accum_out=rowsum[:, sm:sm + 1])
                rrow = sums.tile([P, SI], FP32, tag="rrow")
                nc.vector.reciprocal(out=rrow, in_=rowsum)

                # ---- transpose E (PE, bf16); col-sums via accum_out -----
                eat = epool.tile([P, SI, L], BF16, tag="eat")      # E[t, s]
                colsum = sums.tile([P, SI], FP32, tag="colsum")
                for tm in range(SI):
                    pt = ps1.tile([P, L], FP32, tag="ps1")
                    for sk in range(SI):
                        nc.tensor.matmul(
                            pt[:, sk * P:(sk + 1) * P],
                            lhsT=ea[:, sk, tm * P:(tm + 1) * P],
                            rhs=ident, start=True, stop=True,
                        )
                    nc.scalar.activation(out=eat[:, tm, :], in_=pt,
                                         func=AFT.Copy,
                                         accum_out=colsum[:, tm:tm + 1])
                rcol = sums.tile([P, SI], FP32, tag="rcol")
                nc.vector.reciprocal(out=rcol, in_=colsum)

                # ---- features, a/b groups interleaved so the DVE
                #      epilogues stream while the PE fills the next bank --
                #      a_feature[t, e] = (E.T @ a_nat)[t, e] / colsum[t]
                #      b_feature[s, e] = (E @ b_nat)[s, e] / rowsum[s]
                for fm in range(SI):
                    for lhs, rvec, x_nat, ft, use_act in (
                        (ea, rcol, x_bf["a"], a_ft, True),
                        (eat, rrow, x_bf["b"], b_ft, False),
                    ):
                        pts = [ps3.tile([P, NF], FP32, tag="ps3", name=f"psf{nj}") for nj in range(NJ)]
                        for sk in range(SI):
                            for nj in range(NJ):
                                nc.tensor.matmul(
                                    pts[nj],
                                    lhsT=lhs[:, sk, fm * P:(fm + 1) * P],
                                    rhs=x_nat[:, sk, nj * NF:(nj + 1) * NF],
                                    start=(sk == 0), stop=(sk == SI - 1),
                                )
                        ot = outp.tile([P, EH], FP16, tag="out")
                        for nj in range(NJ):
                            # a-groups evacuate on ACT, b-groups on DVE, so
                            # the two engines drain banks concurrently
                            if use_act:
                                nc.scalar.activation(
                                    out=ot[:, nj * NF:(nj + 1) * NF],
                                    in_=pts[nj], func=AFT.Copy,
                                    scale=rvec[:, fm:fm + 1])
                            else:
                                nc.vector.tensor_scalar_mul(
                                    out=ot[:, nj * NF:(nj + 1) * NF],
                                    in0=pts[nj], scalar1=rvec[:, fm:fm + 1])
                        nc.sync.dma_start(
                            out=ft[ib, fm * P:(fm + 1) * P, :], in_=ot)

    nc.compile()
    return nc


@functools.lru_cache(maxsize=4)
def _build_cached(temp: float, repeat: int = 1, body: int = 1) -> bacc.Bacc:
    return _build(temp, repeat, body)


def _prep_inputs(inputs: dict) -> list[dict]:
    a_f32 = np.asarray(inputs["a_inputs"], dtype=np.float32)
    b_f32 = np.asarray(inputs["b_inputs"], dtype=np.float32)
    a_bf = np.ascontiguousarray(a_f32.astype(ml_dtypes.bfloat16))
    b_bf = np.ascontiguousarray(b_f32.astype(ml_dtypes.bfloat16))

    def t8(x):
        # [B, L, EH] -> fp8 [B, EJ, P, L] with [b, ej, p, s] = x[b, s, ej*P+p]
        q = x.astype(ml_dtypes.float8_e4m3)
        return np.ascontiguousarray(
            q.reshape(B, L, EJ, P).transpose(0, 2, 3, 1))

    a8t = t8(a_f32)
    b8t = t8(b_f32)
    W1q = np.ascontiguousarray(np.clip(
        np.asarray(inputs["W1"], dtype=np.float32) * WSCALE, -240, 240
    ).astype(ml_dtypes.float8_e4m3))
    W2q = np.ascontiguousarray(np.clip(
        np.asarray(inputs["W2"], dtype=np.float32) * WSCALE, -240, 240
    ).astype(ml_dtypes.float8_e4m3))
    b1s = np.ascontiguousarray(np.asarray(inputs["b1"], np.float32) * OSCALE)
    b2s = np.ascontiguousarray(np.asarray(inputs["b2"], np.float32) * OSCALE)

    in_maps = []
    for c in range(N_CORES):
        sl = slice(c * BPC, (c + 1) * BPC)
        in_maps.append({
            "a_bf": a_bf[sl],
            "b_bf": b_bf[sl],
            "a8t": a8t[sl],
            "b8t": b8t[sl],
            "W1q": W1q, "b1s": b1s, "W2q": W2q, "b2s": b2s,
        })
    return in_maps


def _run(inputs: dict, trace: bool = False):
    temp = float(np.asarray(inputs["temperature"]))
    nc = _build_cached(temp)
    in_maps = _prep_inputs(inputs)
    res = run_bass_kernel_spmd(nc, in_maps, list(range(N_CORES)), trace=trace)
    a_feat = np.concatenate(
        [res.results[c]["a_feature"].astype(np.float32) for c in range(N_CORES)], axis=0)
    b_feat = np.concatenate(
        [res.results[c]["b_feature"].astype(np.float32) for c in range(N_CORES)], axis=0)
    return (a_feat, b_feat), res


def kernel(a_inputs, a_mask, b_inputs, b_mask, W1, b1, W2, b2, temperature):
    (a_feat, b_feat), _ = _run({
        "a_inputs": a_inputs, "b_inputs": b_inputs,
        "W1": W1, "b1": b1, "W2": W2, "b2": b2,
        "temperature": temperature,
    })
    return (a_feat, b_feat)
